# revision 1
# baseline (speedup 1.0000x reference)
"""DeepGravityEasy segment-softmax kernel for Trainium2 (8 NeuronCores).

Pipeline per core (rows sharded across cores, MLP weights replicated):
  Phase A: x --(DMA)--> SBUF, PE-transpose to feature-major, 3-layer MLP on PE
           (float32r matmuls), relu via ScalarE activation, dense logits block
           built with the W3-column trick (tile q -> partition q of the logits
           PSUM block), exp fused with the +b3 bias on ScalarE.
  Phase B: segmented sum into 4096 bins via one-hot matmuls on PE
           (lhsT = e-weighted 32-wide hi one-hot, rhs = 128-wide lo one-hot in
           bf16), PSUM-accumulated; AllReduce bins across the 8 cores.
  Phase C: reciprocal of bins, table replicated to all partitions, per-element
           gather via GPSIMD ap_gather (16x redundant within each Q7 core
           group), diagonal selection, multiply with e, DMA out.

Softmax max-subtraction is skipped: it cancels exactly in exact arithmetic and
the logits of this model are O(1) (verified against the reference), so exp
never overflows.
"""
import sys

sys.path.insert(0, "/opt/trn_rl_repo")

import numpy as np
from contextlib import ExitStack
from dataclasses import dataclass

import concourse.bass as bass
import concourse.bacc as bacc
import concourse.tile as tile
import concourse.mybir as mybir
import bass_rust
from concourse._compat import with_exitstack

AF = mybir.ActivationFunctionType
ALU = mybir.AluOpType
dt = mybir.dt

P = 128
D = 64
TILE = 512
NB = 4096  # num origin bins
ADD_DEP = bass_rust.add_dep_helper


@dataclass
class Cfg:
    sb_tiles: int = 128   # logit tiles per superblock (= partitions used)
    n_sb: int = 4         # superblocks per core
    n_cores: int = 8
    gather_chunk: int = 512   # columns per ap_gather chunk (per superblock)
    diag_mode: str = "dve"    # "dve" | "dma"
    use_f32r: bool = True

    @property
    def m_loc(self):
        return self.n_sb * self.sb_tiles * TILE

    @property
    def ncol(self):
        return self.n_sb * TILE


def _mmdt(cfg):
    return dt.float32r if cfg.use_f32r else dt.float32


@with_exitstack
def build_kernel(ctx: ExitStack, tc: tile.TileContext, io: dict, cfg: Cfg):
    nc = tc.nc
    SBT = cfg.sb_tiles
    NCOL = cfg.ncol
    U = SBT // 2  # pairs per superblock

    x_ap = io["x"].ap()            # (M_LOC, 64) f32
    ids_ap = io["ids"].ap()        # (M_LOC,) int32
    out_ap = io["out"].ap()        # (M_LOC,) f32
    ident_ap = io["ident"].ap()    # (128,128) f32
    iota128_ap = io["iota128"].ap()  # (128,128) f32
    iota32_ap = io["iota32"].ap()    # (128,32) f32
    sel16_ap = io["sel16"].ap()      # (128,16) f32  one-hot of p%16
    w1_ap = io["w1blk"].ap()       # (128,128) blockdiag W1
    w2_ap = io["w2blk"].ap()       # (128,128) blockdiag W2
    w3_ap = io["w3blk"].ap()       # (128,127) W3 at (0:64,63) and (64:128,64)
    b1_ap = io["b1dup"].ap()       # (128,1) f32
    b2_ap = io["b2dup"].ap()       # (128,1) f32
    b3_ap = io["b3dup"].ap()       # (128,1) f32

    # DRAM views for the fancy loads
    xr = x_ap.rearrange(
        "(b u h c p) d -> b u h p c d", b=cfg.n_sb, u=U, h=2, c=4, p=128
    )
    idsr = ids_ap.rearrange("(b q f) -> q b f", b=cfg.n_sb, q=SBT, f=TILE)
    outr = out_ap.rearrange("(b q f) -> q b f", b=cfg.n_sb, q=SBT, f=TILE)

    # ---------------- persistent SBUF ----------------
    pers = ctx.enter_context(tc.tile_pool(name="pers", bufs=1))
    MMDT = _mmdt(cfg)
    ident = pers.tile([P, P], MMDT)
    iota128 = pers.tile([SBT, 128], dt.float32)
    iota32 = pers.tile([SBT, 32], dt.float32)
    sel16 = pers.tile([SBT, 16], dt.float32)
    w1 = pers.tile([P, P], MMDT)
    w2 = pers.tile([P, P], MMDT)
    w3 = pers.tile([P, 127], MMDT)
    b1 = pers.tile([P, 1], dt.float32)
    b2 = pers.tile([P, 1], dt.float32)
    b3 = pers.tile([P, 1], dt.float32)
    nc.sync.dma_start(ident[:], ident_ap)
    nc.sync.dma_start(iota128[:], iota128_ap[:SBT])
    nc.sync.dma_start(iota32[:], iota32_ap[:SBT])
    nc.sync.dma_start(sel16[:], sel16_ap[:SBT])
    nc.sync.dma_start(w1[:], w1_ap)
    nc.sync.dma_start(w2[:], w2_ap)
    nc.sync.dma_start(w3[:], w3_ap)
    nc.sync.dma_start(b1[:], b1_ap)
    nc.sync.dma_start(b2[:], b2_ap)
    nc.sync.dma_start(b3[:], b3_ap)

    e_all = pers.tile([SBT, NCOL], dt.float32)
    ids_i32 = pers.tile([SBT, NCOL], dt.int32)
    ids_i16 = pers.tile([SBT, NCOL], dt.int16)

    nc.sync.dma_start(
        ids_i32[:].rearrange("q (b f) -> q b f", b=cfg.n_sb), idsr
    )
    nc.vector.tensor_copy(ids_i16[:], ids_i32[:])

    # ---------------- phase A: MLP + logits + exp ----------------
    # Each "pair" u covers tiles (2u, 2u+1) = 1024 rows. The transpose stacks
    # tile-2u features on partitions 0-63 and tile-2u+1 on 64-127, so L1/L2
    # run as single K=128 matmuls against block-diagonal weights
    # [[W,0],[0,W]] and L3 as a K=128 matmul against a two-column W3 block
    # (tile q -> logits partition q%64, PSUM bank q//64). float32r keeps the
    # moving operand at 1 cycle/row (N=512) with no tile_position use, which
    # fp32r does not support.
    nbank = (SBT + 63) // 64
    with ExitStack() as pa:
        xp_pool = pa.enter_context(tc.tile_pool(name="xp", bufs=3))
        xt_pool = pa.enter_context(tc.tile_pool(name="xt", bufs=3))
        h_pool = pa.enter_context(tc.tile_pool(name="h", bufs=3))
        et_pool = pa.enter_context(tc.tile_pool(name="et", bufs=2))
        ps_pool = pa.enter_context(tc.tile_pool(name="psA", bufs=2, space="PSUM"))
        pslog_pool = pa.enter_context(
            tc.tile_pool(name="psL", bufs=1, space="PSUM")
        )
        for B in range(cfg.n_sb):
            logbanks = []
            for i in range(nbank):
                logbank = pslog_pool.tile(
                    [64, TILE], dt.float32, tag=f"log{i}", name=f"logbank{i}"
                )
                logbanks.append(logbank)
            for u in range(U):
                q0 = 2 * u
                xpair = xp_pool.tile([P, 4, 2, D], MMDT, tag="xpair")
                nc.sync.dma_start(xpair[:, :, 0, :], xr[B, u, 0])
                nc.sync.dma_start(xpair[:, :, 1, :], xr[B, u, 1])
                xT_ps = ps_pool.tile([P, TILE], MMDT, tag="xT")
                for k in range(4):
                    nc.tensor.transpose(
                        xT_ps[:, 128 * k : 128 * (k + 1)],
                        xpair[:, k].rearrange("p h d -> p (h d)"),
                        ident[:],
                    )
                xT = xt_pool.tile([P, TILE], MMDT, tag="xT_sb")
                nc.vector.tensor_copy(xT[:], xT_ps[:])
                h1_ps = ps_pool.tile([P, TILE], dt.float32, tag="h1")
                nc.tensor.matmul(h1_ps[:], w1[:], xT[:], start=True, stop=True)
                h1 = h_pool.tile([P, TILE], MMDT, tag="h1_sb")
                nc.scalar.activation(h1[:], h1_ps[:], AF.Relu, bias=b1[:], scale=1.0)
                h2_ps = ps_pool.tile([P, TILE], dt.float32, tag="h2")
                nc.tensor.matmul(h2_ps[:], w2[:], h1[:], start=True, stop=True)
                h2 = h_pool.tile([P, TILE], MMDT, tag="h2_sb")
                nc.scalar.activation(h2[:], h2_ps[:], AF.Relu, bias=b2[:], scale=1.0)
                # L3: tiles (2u, 2u+1) -> partitions (q0%64, q0%64+1) of bank
                bank = q0 // 64
                c = q0 % 64
                upb = min(U, 32 * (bank + 1)) - 32 * bank  # pairs in this bank
                first = c == 0
                last = (c == 62) or (u == U - 1)
                nc.tensor.matmul(
                    logbanks[bank][:],
                    w3[:, 63 - c : 127 - c],
                    h2[:],
                    start=first, stop=last,
                )
            for bank in range(nbank):
                rows = min(64, SBT - 64 * bank)
                e_tmp = et_pool.tile([64, TILE], dt.float32, tag="e_tmp")
                nc.scalar.activation(
                    e_tmp[0:rows, :],
                    logbanks[bank][0:rows, :],
                    AF.Exp,
                    bias=b3[0:rows],
                    scale=1.0,
                )
                # reassemble into e_all partitions [64*bank, 64*bank+rows)
                nc.sync.dma_start(
                    e_all[64 * bank : 64 * bank + rows,
                          B * TILE : (B + 1) * TILE],
                    e_tmp[0:rows, :],
                )

    # ---------------- phase B: binning ----------------
    # e is split e = e_hi + e_lo (both bf16) so the one-hot matmuls can run in
    # bf16 while the PSUM accumulation keeps ~16-bit per-element precision.
    with ExitStack() as pb:
        pbp = pb.enter_context(tc.tile_pool(name="pbp", bufs=1))
        lo_f = pbp.tile([SBT, NCOL], dt.float32)
        hi_f = pbp.tile([SBT, NCOL], dt.float32)
        tmp_i = pbp.tile([SBT, NCOL], dt.int32)
        e_hi = pbp.tile([SBT, NCOL], dt.bfloat16)
        e_lo = pbp.tile([SBT, NCOL], dt.float32)
        nc.vector.tensor_scalar(
            tmp_i[:], ids_i32[:], 127, None, op0=ALU.bitwise_and
        )
        nc.vector.tensor_copy(lo_f[:], tmp_i[:])
        nc.vector.tensor_scalar(
            tmp_i[:], ids_i32[:], 7, None, op0=ALU.logical_shift_right
        )
        nc.vector.tensor_copy(hi_f[:], tmp_i[:])
        nc.vector.tensor_copy(e_hi[:], e_all[:])
        nc.vector.tensor_tensor(
            out=e_lo[:], in0=e_all[:], in1=e_hi[:], op=ALU.subtract
        )
        mask_pool = pb.enter_context(tc.tile_pool(name="masks", bufs=4))
        psb_pool = pb.enter_context(tc.tile_pool(name="psB", bufs=1, space="PSUM"))
        bins_ps = psb_pool.tile([64, 128], dt.float32)
        for col in range(NCOL):
            A = mask_pool.tile([SBT, 128], dt.bfloat16, tag="A")
            H2 = mask_pool.tile([SBT, 64], dt.bfloat16, tag="H")
            nc.vector.tensor_scalar(
                A[:], iota128[:], lo_f[:, col : col + 1], None, op0=ALU.is_equal
            )
            nc.vector.tensor_scalar(
                H2[:, 0:32], iota32[:], hi_f[:, col : col + 1],
                e_all[:, col : col + 1], op0=ALU.is_equal, op1=ALU.mult,
            )
            nc.vector.tensor_scalar(
                H2[:, 32:64], iota32[:], hi_f[:, col : col + 1],
                e_lo[:, col : col + 1], op0=ALU.is_equal, op1=ALU.mult,
            )
            nc.tensor.matmul(
                bins_ps[:], H2[:], A[:],
                start=(col == 0), stop=(col == NCOL - 1),
            )
        # combine hi+lo partial bins: comb64.T @ bins64 adds rows k and k+32
        bins64 = pers.tile([64, 128], dt.float32)
        nc.vector.tensor_copy(bins64[:], bins_ps[:])
        comb = pers.tile([64, 32], dt.float32)
        nc.sync.dma_start(comb[:], io["comb64"].ap())
        binsC_ps = psb_pool.tile([32, 128], dt.float32, tag="binsC")
        nc.tensor.matmul(binsC_ps[:], comb[:], bins64[:], start=True, stop=True)
        bins_sb = pers.tile([32, 128], dt.float32)
        nc.vector.tensor_copy(bins_sb[:], binsC_ps[:])

    # ---------------- all-reduce bins across cores ----------------
    binsred_sb = pers.tile([32, 128], dt.float32)
    if cfg.n_cores > 1:
        bins_in = io["bins_in"].ap()
        bins_out = io["bins_out"].ap()
        nc.sync.dma_start(bins_in, bins_sb[:])
        nc.gpsimd.collective_compute(
            "AllReduce",
            ALU.add,
            replica_groups=[list(range(cfg.n_cores))],
            ins=[bins_in],
            outs=[bins_out],
        )
        nc.sync.dma_start(binsred_sb[:], bins_out)
    else:
        nc.vector.tensor_copy(binsred_sb[:], bins_sb[:])

    # tiny additive guard: empty bins (possible at small M) give 1/eps, not inf
    nc.vector.tensor_scalar(
        binsred_sb[:], binsred_sb[:], 1e-30, None, op0=ALU.add
    )
    invd = pers.tile([32, 128], dt.float32)
    nc.vector.reciprocal(invd[:], binsred_sb[:])
    invd_row = pers.tile([1, NB], dt.float32)
    nc.sync.dma_start(invd_row[:], invd[:])
    T_sb = pers.tile([SBT, NB], dt.float32)
    nc.gpsimd.partition_broadcast(T_sb[:], invd_row[:])

    # ---------------- phase C: gather + final ----------------
    CH = cfg.gather_chunk
    out_all = pers.tile([SBT, NCOL], dt.float32)
    with ExitStack() as pc:
        gr_pool = pc.enter_context(tc.tile_pool(name="gred", bufs=1))
        for c0 in range(0, NCOL, CH):
            g_red = gr_pool.tile([SBT, CH * 16], dt.float32, tag="gred")
            nc.gpsimd.ap_gather(
                g_red[:], T_sb[:], ids_i16[:, c0 : c0 + CH],
                channels=SBT, num_elems=NB, d=1, num_idxs=CH * 16,
            )
            g3 = g_red[:].rearrange("p (f r) -> p f r", r=16)
            if cfg.diag_mode == "dve":
                prod = gr_pool.tile([SBT, CH * 16], dt.float32, tag="prod")
                nc.vector.tensor_tensor(
                    out=prod[:].rearrange("p (f r) -> p f r", r=16),
                    in0=g3,
                    in1=sel16[:, None, :].to_broadcast([SBT, CH, 16]),
                    op=ALU.mult,
                )
                gsel = gr_pool.tile([SBT, CH], dt.float32, tag="gsel")
                nc.vector.tensor_reduce(
                    out=gsel[:, :, None],
                    in_=prod[:].rearrange("p (f r) -> p f r", r=16),
                    axis=mybir.AxisListType.X,
                    op=ALU.add,
                )
                nc.vector.tensor_tensor(
                    out=out_all[:, c0 : c0 + CH],
                    in0=gsel[:],
                    in1=e_all[:, c0 : c0 + CH],
                    op=ALU.mult,
                )
            else:  # "dma": multiply e in redundant space, strided-DMA diagonal
                prod = gr_pool.tile([SBT, CH * 16], dt.float32, tag="prod")
                mul = nc.vector.tensor_tensor(
                    out=prod[:].rearrange("p (f r) -> p f r", r=16),
                    in0=g3,
                    in1=e_all[:, c0 : c0 + CH, None].to_broadcast([SBT, CH, 16]),
                    op=ALU.mult,
                )
                pr3 = prod[:].rearrange("p (f r) -> p f r", r=16)
                B0, f0 = divmod(c0, TILE)
                for qq in range(16):
                    dst = outr[qq::16, B0, f0 : f0 + CH]
                    dmai = nc.sync.dma_start(dst, pr3[qq::16, :, qq])
                    ADD_DEP(dmai.ins, mul.ins, sync=True, reason="diag")
    if cfg.diag_mode == "dve":
        nc.sync.dma_start(
            outr, out_all[:].rearrange("q (b f) -> q b f", b=cfg.n_sb)
        )


def host_consts(W1, b1, W2, b2, W3, b3):
    ident = np.eye(P, dtype=np.float32)
    iota128 = np.tile(np.arange(128, dtype=np.float32), (P, 1))
    iota32 = np.tile(np.arange(32, dtype=np.float32), (P, 1))
    sel16 = np.zeros((P, 16), np.float32)
    sel16[np.arange(P), np.arange(P) % 16] = 1.0
    def blockdiag(W):
        Z = np.zeros((64, 64), np.float32)
        return np.block([[W, Z], [Z, W]]).astype(np.float32)

    w3blk = np.zeros((128, 127), np.float32)
    w3blk[0:64, 63] = W3[:, 0]
    w3blk[64:128, 64] = W3[:, 0]
    comb64 = np.vstack([np.eye(32, dtype=np.float32)] * 2)
    return {
        "comb64": comb64,
        "ident": ident,
        "iota128": iota128,
        "iota32": iota32,
        "sel16": sel16,
        "w1blk": blockdiag(np.asarray(W1, np.float32)),
        "w2blk": blockdiag(np.asarray(W2, np.float32)),
        "w3blk": w3blk,
        "b1dup": np.concatenate([b1, b1])[:, None].astype(np.float32),
        "b2dup": np.concatenate([b2, b2])[:, None].astype(np.float32),
        "b3dup": np.tile(np.float32(b3[0]), (P, 1)).astype(np.float32),
    }


def make_module(cfg: Cfg):
    nc = bacc.Bacc(
        "TRN2",
        target_bir_lowering=False,
        debug=False,
        enable_asserts=True,
        num_devices=cfg.n_cores,
    )
    io = {}
    mmdt = _mmdt(cfg)
    io["x"] = nc.dram_tensor("x", (cfg.m_loc, D), mmdt, kind="ExternalInput")
    io["ids"] = nc.dram_tensor("ids", (cfg.m_loc,), dt.int32, kind="ExternalInput")
    for name, shape, d in [
        ("ident", (P, P), mmdt), ("iota128", (P, 128), dt.float32),
        ("iota32", (P, 32), dt.float32), ("sel16", (P, 16), dt.float32),
        ("comb64", (64, 32), dt.float32),
        ("w1blk", (P, P), mmdt), ("w2blk", (P, P), mmdt),
        ("w3blk", (P, 127), mmdt), ("b1dup", (P, 1), dt.float32),
        ("b2dup", (P, 1), dt.float32), ("b3dup", (P, 1), dt.float32),
    ]:
        io[name] = nc.dram_tensor(name, shape, d, kind="ExternalInput")
    io["out"] = nc.dram_tensor("out", (cfg.m_loc,), dt.float32, kind="ExternalOutput")
    if cfg.n_cores > 1:
        io["bins_in"] = nc.dram_tensor("bins_in", (32, 128), dt.float32, kind="Internal")
        io["bins_out"] = nc.dram_tensor("bins_out", (32, 128), dt.float32, kind="Internal")
    with tile.TileContext(nc) as tc:
        build_kernel(tc, io, cfg)
    nc.compile()
    return nc


_CACHE = {}


def _get_module(cfg: Cfg):
    key = (cfg.sb_tiles, cfg.n_sb, cfg.n_cores, cfg.gather_chunk, cfg.diag_mode,
           cfg.use_f32r)
    if key not in _CACHE:
        _CACHE[key] = make_module(cfg)
    return _CACHE[key]


def run_spmd(cfg: Cfg, x, origin_ids, W1, b1, W2, b2, W3, b3, **run_kw):
    """x: (M, 64) fp32; origin_ids: (M,) int32. Returns (out (M,), results)."""
    from concourse.bass_utils import run_bass_kernel_spmd

    M = x.shape[0]
    assert M == cfg.m_loc * cfg.n_cores, (M, cfg.m_loc, cfg.n_cores)
    nc = _get_module(cfg)
    consts = host_consts(W1, b1, W2, b2, W3, b3)
    in_maps = []
    for c in range(cfg.n_cores):
        sl = slice(c * cfg.m_loc, (c + 1) * cfg.m_loc)
        m = {"x": np.ascontiguousarray(x[sl]),
             "ids": np.ascontiguousarray(origin_ids[sl])}
        m.update(consts)
        in_maps.append(m)
    res = run_bass_kernel_spmd(nc, in_maps, core_ids=list(range(cfg.n_cores)),
                               **run_kw)
    out = np.concatenate([res.results[c]["out"] for c in range(cfg.n_cores)])
    return out, res


def kernel(**inputs) -> np.ndarray:
    cfg = Cfg()
    out, _ = run_spmd(
        cfg,
        np.asarray(inputs["x"], dtype=np.float32),
        np.asarray(inputs["origin_ids"], dtype=np.int32),
        np.asarray(inputs["W1"], dtype=np.float32),
        np.asarray(inputs["b1"], dtype=np.float32),
        np.asarray(inputs["W2"], dtype=np.float32),
        np.asarray(inputs["b2"], dtype=np.float32),
        np.asarray(inputs["W3"], dtype=np.float32),
        np.asarray(inputs["b3"], dtype=np.float32),
    )
    return out



# revision 2
# speedup vs baseline: 2.6183x; 2.6183x over previous
"""DeepGravityEasy segment-softmax kernel for Trainium2 (8 NeuronCores).

v2 — optimized for end-to-end time. The dominant cost of v1 was pushing
512 MB of fp32 x through the host->device link plus host-side copies and
per-call jit rebuilds. v2:

  Host:  x -> fp16 (threaded astype, halves the big transfer), ids -> int16
         reordered into the device tile layout, weights -> fp16 block-diag.
         Device-side layout avoids every per-core slice/concat copy: the
         jitted shard_map callable (built once, cached) takes the full
         concatenated arrays directly.
  Device (per core, 262144 rows):
    Phase A: per 1024-row chunk, one XBAR DMA-transpose loads x directly
         feature-major ([128, 512] fp16: partitions 0:63 = features of even
         rows, 64:127 = odd rows), then a 3-matmul fp16 MLP with fused
         relu/bias on ScalarE. L3 uses the W3-column trick to pack logits of
         32 chunks into one [64, 512] PSUM bank; exp (fused +b3) writes
         straight into e_all.
    Phase B: segmented sums into 4096 bins as 64x64 one-hot matmuls
         (bf16, single pass - no hi/lo split), PSUM-accumulated over all
         2048 columns; AllReduce the [64,64] bins across the 8 cores.
    Phase C: reciprocal, broadcast table, GPSIMD ap_gather (16x redundant),
         DVE diagonal select, multiply with e, contiguous DMA out. Host
         inverse-permutes the [8*128, 2048] result to row order.

Max-subtraction is skipped: logits of this model are O(1) (verified), so
exp never overflows and it cancels exactly otherwise.
"""
import sys

sys.path.insert(0, "/opt/trn_rl_repo")

import numpy as np
import ml_dtypes
from contextlib import ExitStack
from dataclasses import dataclass
from concurrent.futures import ThreadPoolExecutor

import jax
import concourse.bass as bass
import concourse.bacc as bacc
import concourse.tile as tile
import concourse.mybir as mybir
from concourse._compat import with_exitstack

AF = mybir.ActivationFunctionType
ALU = mybir.AluOpType
dt = mybir.dt

P = 128
D = 64
NB = 4096           # num origin bins
N_CORES = 8
M_FULL = 2097152
M_LOC = M_FULL // N_CORES   # 262144 rows per core
CHP = 512                   # pair-columns per chunk (1024 rows)
NCHUNK = M_LOC // (2 * CHP)  # 256 chunks per core
SLOTS = 32                  # chunks accumulated per logits PSUM bank
NBANK = NCHUNK // SLOTS     # 8 banks -> e_all [128, 2048]
NCOL = M_LOC // P           # 2048


@dataclass
class Cfg:
    n_cores: int = N_CORES
    gather_chunk: int = 512   # columns per ap_gather chunk


@with_exitstack
def build_kernel(ctx: ExitStack, tc: tile.TileContext, io: dict, cfg: Cfg):
    nc = tc.nc

    x_ap = io["x"].ap()          # (M_LOC, 64) f16
    ids_ap = io["ids_t"].ap()    # (128, 2048) i16, pre-permuted on host
    out_ap = io["out"].ap()      # (128, 2048) f32

    # chunk ch covers rows [1024*ch, 1024*ch+1024) viewed as (512, 128):
    # row pair i side by side -> transposing gives partitions 0:64 = features
    # of even rows, 64:128 = odd rows.
    xr = x_ap.rearrange("(ch i two) d -> ch i (two d)", ch=NCHUNK, i=CHP, two=2)

    # ---------------- persistent SBUF ----------------
    pers = ctx.enter_context(tc.tile_pool(name="pers", bufs=1))
    w1f = pers.tile([P, P], dt.float16)
    w2f = pers.tile([P, P], dt.float16)
    w3f = pers.tile([P, 127], dt.float16)
    b1d = pers.tile([P, 1], dt.float32)
    b2d = pers.tile([P, 1], dt.float32)
    b3d = pers.tile([64, 1], dt.float32)
    iota64 = pers.tile([P, 64], dt.bfloat16)
    sel16 = pers.tile([P, 16], dt.float32)
    for name, t in [("w1f", w1f), ("w2f", w2f), ("w3f", w3f), ("b1d", b1d),
                    ("b2d", b2d), ("b3d", b3d), ("iota64", iota64),
                    ("sel16", sel16)]:
        nc.sync.dma_start(t[:], io[name].ap())

    e_all = pers.tile([P, NCOL], dt.float32)
    ids_t = pers.tile([P, NCOL], dt.int16)
    out_all = pers.tile([P, NCOL], dt.float32)
    nc.sync.dma_start(ids_t[:], ids_ap)

    # ---------------- phase A: MLP + logits + exp ----------------
    with ExitStack() as pa:
        xt_pool = pa.enter_context(tc.tile_pool(name="xt", bufs=3))
        h_pool = pa.enter_context(tc.tile_pool(name="h", bufs=2))
        ps_pool = pa.enter_context(tc.tile_pool(name="psA", bufs=2, space="PSUM"))
        pslog = pa.enter_context(tc.tile_pool(name="psL", bufs=2, space="PSUM"))
        logbank = None
        for ch in range(NCHUNK):
            b, s = divmod(ch, SLOTS)
            xT = xt_pool.tile([P, CHP], dt.float16, tag="xT")
            nc.sync.dma_start_transpose(xT[:], xr[ch])
            h1_ps = ps_pool.tile([P, CHP], dt.float32, tag="h1")
            nc.tensor.matmul(h1_ps[:], w1f[:], xT[:], start=True, stop=True)
            h1 = h_pool.tile([P, CHP], dt.float16, tag="h1s")
            nc.scalar.activation(h1[:], h1_ps[:], AF.Relu, bias=b1d[:], scale=1.0)
            h2_ps = ps_pool.tile([P, CHP], dt.float32, tag="h2")
            nc.tensor.matmul(h2_ps[:], w2f[:], h1[:], start=True, stop=True)
            h2 = h_pool.tile([P, CHP], dt.float16, tag="h2s")
            nc.scalar.activation(h2[:], h2_ps[:], AF.Relu, bias=b2d[:], scale=1.0)
            # L3: chunk ch -> partitions (2s, 2s+1) of bank b
            if s == 0:
                logbank = pslog.tile([64, CHP], dt.float32, tag="log")
            c = 2 * s
            nc.tensor.matmul(
                logbank[:], w3f[:, 63 - c : 127 - c], h2[:],
                start=(s == 0), stop=(s == SLOTS - 1),
            )
            if s == SLOTS - 1:
                B, q1 = divmod(b, 2)
                nc.scalar.activation(
                    e_all[64 * q1 : 64 * q1 + 64, B * CHP : (B + 1) * CHP],
                    logbank[:], AF.Exp, bias=b3d[:], scale=1.0,
                )

    # ---------------- phase B: binning (64 hi x 64 lo one-hot matmuls) ----
    with ExitStack() as pb:
        pbp = pb.enter_context(tc.tile_pool(name="pbp", bufs=1))
        lo6 = pbp.tile([P, NCOL], dt.float32)
        hi6 = pbp.tile([P, NCOL], dt.float32)
        tmp = pbp.tile([P, NCOL], dt.int16)
        nc.vector.tensor_scalar(tmp[:], ids_t[:], 63, None, op0=ALU.bitwise_and)
        nc.vector.tensor_copy(lo6[:], tmp[:])
        nc.vector.tensor_scalar(tmp[:], ids_t[:], 6, None,
                                op0=ALU.logical_shift_right)
        nc.vector.tensor_copy(hi6[:], tmp[:])
        mask_pool = pb.enter_context(tc.tile_pool(name="masks", bufs=4))
        psb = pb.enter_context(tc.tile_pool(name="psB", bufs=1, space="PSUM"))
        bins_ps = psb.tile([64, 64], dt.float32)
        for col in range(NCOL):
            A = mask_pool.tile([P, 64], dt.bfloat16, tag="A")
            H = mask_pool.tile([P, 64], dt.bfloat16, tag="H")
            nc.vector.tensor_scalar(
                A[:], iota64[:], lo6[:, col : col + 1], None, op0=ALU.is_equal
            )
            nc.vector.tensor_scalar(
                H[:], iota64[:], hi6[:, col : col + 1],
                e_all[:, col : col + 1], op0=ALU.is_equal, op1=ALU.mult,
            )
            nc.tensor.matmul(
                bins_ps[:], H[:], A[:],
                start=(col == 0), stop=(col == NCOL - 1),
            )
        bins_sb = pers.tile([64, 64], dt.float32)
        nc.vector.tensor_copy(bins_sb[:], bins_ps[:])

    # ---------------- all-reduce bins across cores ----------------
    binsred = pers.tile([64, 64], dt.float32)
    if cfg.n_cores > 1:
        bins_in = io["bins_in"].ap()
        bins_out = io["bins_out"].ap()
        nc.sync.dma_start(bins_in, bins_sb[:])
        nc.gpsimd.collective_compute(
            "AllReduce", ALU.add,
            replica_groups=[list(range(cfg.n_cores))],
            ins=[bins_in], outs=[bins_out],
        )
        nc.sync.dma_start(binsred[:], bins_out)
    else:
        nc.vector.tensor_copy(binsred[:], bins_sb[:])

    # empty bins give 1/eps, not inf
    nc.vector.tensor_scalar(binsred[:], binsred[:], 1e-30, None, op0=ALU.add)
    invd = pers.tile([64, 64], dt.float32)
    nc.vector.reciprocal(invd[:], binsred[:])
    invd_row = pers.tile([1, NB], dt.float32)
    nc.sync.dma_start(invd_row[:], invd[:])
    T_sb = pers.tile([P, NB], dt.float32)
    nc.gpsimd.partition_broadcast(T_sb[:], invd_row[:])

    # ---------------- phase C: gather + final ----------------
    CH = cfg.gather_chunk
    with ExitStack() as pc:
        gr = pc.enter_context(tc.tile_pool(name="gred", bufs=1))
        for c0 in range(0, NCOL, CH):
            g_red = gr.tile([P, CH * 16], dt.float32, tag="gred")
            nc.gpsimd.ap_gather(
                g_red[:], T_sb[:], ids_t[:, c0 : c0 + CH],
                channels=P, num_elems=NB, d=1, num_idxs=CH * 16,
            )
            prod = gr.tile([P, CH * 16], dt.float32, tag="prod")
            nc.vector.tensor_tensor(
                out=prod[:].rearrange("p (f r) -> p f r", r=16),
                in0=g_red[:].rearrange("p (f r) -> p f r", r=16),
                in1=sel16[:, None, :].to_broadcast([P, CH, 16]),
                op=ALU.mult,
            )
            gsel = gr.tile([P, CH], dt.float32, tag="gsel")
            nc.vector.tensor_reduce(
                out=gsel[:, :, None],
                in_=prod[:].rearrange("p (f r) -> p f r", r=16),
                axis=mybir.AxisListType.X, op=ALU.add,
            )
            nc.vector.tensor_tensor(
                out=out_all[:, c0 : c0 + CH],
                in0=gsel[:], in1=e_all[:, c0 : c0 + CH], op=ALU.mult,
            )
    nc.sync.dma_start(out_ap, out_all[:])


def host_consts(W1, b1, W2, b2, W3, b3):
    def blockdiag(W):
        Z = np.zeros((64, 64), np.float32)
        return np.block([[W, Z], [Z, W]]).astype(np.float16)

    w3blk = np.zeros((128, 127), np.float16)
    w3blk[0:64, 63] = W3[:, 0].astype(np.float16)
    w3blk[64:128, 64] = W3[:, 0].astype(np.float16)
    iota64 = np.tile(np.arange(64, dtype=np.float32), (P, 1)).astype(
        ml_dtypes.bfloat16)
    sel16 = np.zeros((P, 16), np.float32)
    sel16[np.arange(P), np.arange(P) % 16] = 1.0
    return {
        "w1f": blockdiag(np.asarray(W1, np.float32)),
        "w2f": blockdiag(np.asarray(W2, np.float32)),
        "w3f": w3blk,
        "b1d": np.concatenate([b1, b1])[:, None].astype(np.float32),
        "b2d": np.concatenate([b2, b2])[:, None].astype(np.float32),
        "b3d": np.tile(np.float32(b3[0]), (64, 1)).astype(np.float32),
        "iota64": iota64,
        "sel16": sel16,
    }


def make_module(cfg: Cfg):
    nc = bacc.Bacc(
        "TRN2",
        target_bir_lowering=False,
        debug=False,
        enable_asserts=False,
        num_devices=cfg.n_cores,
    )
    io = {}
    io["x"] = nc.dram_tensor("x", (M_LOC, D), dt.float16, kind="ExternalInput")
    io["ids_t"] = nc.dram_tensor("ids_t", (P, NCOL), dt.int16, kind="ExternalInput")
    for name, shape, d in [
        ("w1f", (P, P), dt.float16), ("w2f", (P, P), dt.float16),
        ("w3f", (P, 127), dt.float16), ("b1d", (P, 1), dt.float32),
        ("b2d", (P, 1), dt.float32), ("b3d", (64, 1), dt.float32),
        ("iota64", (P, 64), dt.bfloat16), ("sel16", (P, 16), dt.float32),
    ]:
        io[name] = nc.dram_tensor(name, shape, d, kind="ExternalInput")
    io["out"] = nc.dram_tensor("out", (P, NCOL), dt.float32, kind="ExternalOutput")
    if cfg.n_cores > 1:
        io["bins_in"] = nc.dram_tensor("bins_in", (64, 64), dt.float32, kind="Internal")
        io["bins_out"] = nc.dram_tensor("bins_out", (64, 64), dt.float32, kind="Internal")
    with tile.TileContext(nc) as tc:
        build_kernel(tc, io, cfg)
    nc.compile()
    return nc


_EXEC = {}


def _get_exec(cfg: Cfg):
    key = (cfg.n_cores, cfg.gather_chunk)
    if key in _EXEC:
        return _EXEC[key]
    from concourse.bass2jax import (
        install_neuronx_cc_hook, _bass_exec_p, partition_id_tensor)
    from jax.experimental.shard_map import shard_map
    from jax.sharding import Mesh, PartitionSpec

    nc = make_module(cfg)
    install_neuronx_cc_hook()
    partition_name = (
        nc.partition_id_tensor.name if nc.partition_id_tensor else None)
    in_names, out_names, out_avals = [], [], []
    for alloc in nc.m.functions[0].allocations:
        if not isinstance(alloc, mybir.MemoryLocationSet):
            continue
        name = alloc.memorylocations[0].name
        if alloc.kind == "ExternalInput":
            if name != partition_name:
                in_names.append(name)
        elif alloc.kind == "ExternalOutput":
            out_names.append(name)
            out_avals.append(jax.core.ShapedArray(
                tuple(alloc.tensor_shape), mybir.dt.np(alloc.dtype)))
    n_params = len(in_names)
    all_names = list(in_names) + out_names
    if partition_name is not None:
        all_names.append(partition_name)

    def _body(*args):
        operands = list(args)
        if partition_name is not None:
            operands.append(partition_id_tensor())
        outs = _bass_exec_p.bind(
            *operands,
            out_avals=tuple(out_avals),
            in_names=tuple(all_names),
            out_names=tuple(out_names),
            lowering_input_output_aliases=(),
            sim_require_finite=True,
            sim_require_nnan=True,
            nc=nc,
        )
        return tuple(outs)

    devices = jax.devices()[: cfg.n_cores]
    mesh = Mesh(np.asarray(devices), ("core",))
    nin = n_params + len(out_names)
    sharded = jax.jit(
        shard_map(
            _body, mesh=mesh,
            in_specs=(PartitionSpec("core"),) * nin,
            out_specs=(PartitionSpec("core"),) * len(out_names),
            check_rep=False,
        ),
        donate_argnums=tuple(range(n_params, nin)),
        keep_unused=True,
    )
    _EXEC[key] = (sharded, in_names, out_names, out_avals)
    return _EXEC[key]


def _astype_threaded(x, dtype, nthreads=16):
    out = np.empty(x.shape, dtype)
    n = x.shape[0]
    step = -(-n // nthreads)

    def go(k):
        sl = slice(k * step, min((k + 1) * step, n))
        np.copyto(out[sl], x[sl], casting="unsafe")

    with ThreadPoolExecutor(nthreads) as ex:
        list(ex.map(go, range(nthreads)))
    return out


def _permute_ids(ids):
    # row-in-core = B*65536 + q1*32768 + s*1024 + i*2 + par
    # device tile: partition q = 64*q1 + 2*s + par, column j = 512*B + i
    a = ids.astype(np.int16).reshape(N_CORES, 4, 2, 32, 512, 2)
    return np.ascontiguousarray(
        a.transpose(0, 2, 3, 5, 1, 4).reshape(N_CORES * P, NCOL))


def _unpermute_out(o):
    a = o.reshape(N_CORES, 2, 32, 2, 4, 512)
    return np.ascontiguousarray(
        a.transpose(0, 4, 1, 2, 5, 3).reshape(M_FULL))


def _run(cfg: Cfg, x, origin_ids, W1, b1, W2, b2, W3, b3):
    assert x.shape == (M_FULL, D), x.shape
    sharded, in_names, out_names, out_avals = _get_exec(cfg)
    gl = {
        "x": _astype_threaded(np.asarray(x), np.float16),
        "ids_t": _permute_ids(np.asarray(origin_ids)),
    }
    for k, v in host_consts(W1, b1, W2, b2, W3, b3).items():
        gl[k] = np.tile(v, (N_CORES,) + (1,) * (v.ndim - 1))
    args = [gl[n] for n in in_names]
    args += [np.zeros((cfg.n_cores * a.shape[0],) + a.shape[1:], a.dtype)
             for a in out_avals]
    outs = sharded(*args)
    o = np.asarray(outs[out_names.index("out")])
    return _unpermute_out(o.astype(np.float32, copy=False))


class _Res:
    exec_time_ns = None


def run_spmd(cfg: Cfg, x, origin_ids, W1, b1, W2, b2, W3, b3, **run_kw):
    out = _run(cfg, x, origin_ids, W1, b1, W2, b2, W3, b3)
    return out, _Res()


def kernel(**inputs) -> np.ndarray:
    cfg = Cfg()
    out = _run(
        cfg,
        np.asarray(inputs["x"]),
        np.asarray(inputs["origin_ids"]),
        np.asarray(inputs["W1"], dtype=np.float32),
        np.asarray(inputs["b1"], dtype=np.float32),
        np.asarray(inputs["W2"], dtype=np.float32),
        np.asarray(inputs["b2"], dtype=np.float32),
        np.asarray(inputs["W3"], dtype=np.float32),
        np.asarray(inputs["b3"], dtype=np.float32),
    )
    return out
